# revision 1
# baseline (speedup 1.0000x reference)
"""Trainium2 Bass kernel for nn_BasePriorNetwork (4-layer dense transformer).

Sharding: data-parallel over batch (B=8) across 8 NeuronCores; weights
replicated. Activations are kept feature-major ("transposed", [feat, token])
on-chip so every linear is lhsT=W chunks over PSUM accumulation with no
transposes; attention scores are computed keys-major so softmax sums and
attn@v contractions run on the PE via ones-matmuls. All matmuls run in
float32r (full-rate fp32) except the FFN down-projection which is bf16.
"""
import sys, math, os
sys.path.insert(0, '/opt/trn_rl_repo')
import numpy as np
import ml_dtypes

import concourse.bass as bass
import concourse.bacc as bacc
import concourse.tile as tile
from concourse import mybir, bass_isa

f32 = mybir.dt.float32
f32r = mybir.dt.float32r
bf16 = mybir.dt.bfloat16
AF = mybir.ActivationFunctionType
ALU = mybir.AluOpType

B, N, D = 8, 515, 1024
H, DH, L = 8, 64, 4
FF = 4 * D
ROT = 32
NB, MAXD = 32, 128
EPS = 1e-5
NEG = -200.0

NP = 520                      # padded tokens / keys / queries
QT = 260                      # query/free tile (2 per NP)
KCH = [(0, 128), (128, 128), (256, 128), (384, 128), (512, 8)]  # key chunks
TCH = KCH                     # token chunks
NMT = D // 128                # 8 feature tiles per 1024

# q/k head-dim permutation (32-aligned partition bases for rotary):
# rows 0:16 = even rotary dims, 16:32 = pass dims, 32:48 = odd rotary dims,
# 48:64 = pass dims.
PERM = (list(range(0, ROT, 2)) + list(range(ROT, ROT + 16))
        + list(range(1, ROT, 2)) + list(range(ROT + 16, DH)))


def _np_relpos_bias_T(emb):
    """biasT[h, keycol, query] with keys = [tok0..tok514, pad*4, null],
    causal mask and padding folded in. float32."""
    q_pos = np.arange(N)
    k_pos = np.arange(N + 1)
    rel = k_pos[None, :] - q_pos[:, None]
    nn = np.maximum(-rel, 0)
    max_exact = NB // 2
    is_small = nn < max_exact
    nf = np.maximum(nn, 1).astype(np.float32)
    val_large = max_exact + (
        np.log(nf / np.float32(max_exact)).astype(np.float32)
        / np.float32(math.log(MAXD / max_exact)) * np.float32(NB - max_exact)
    ).astype(np.int32)
    val_large = np.minimum(val_large, NB - 1)
    bucket = np.where(is_small, nn, val_large)          # [n, n+1]
    bias = emb[bucket]                                   # [n, n+1, H]
    bias = np.transpose(bias, (2, 0, 1)).astype(np.float32)  # [H, n, n+1]

    out = np.full((H, NP, NP), NEG, np.float32)          # [h, key, query]
    tok = np.transpose(bias[:, :, 1:], (0, 2, 1))        # [H, key=515, q=515]
    jj = np.arange(N)[:, None]
    qq = np.arange(N)[None, :]
    tok = np.where(jj >= qq + 1, np.float32(NEG), tok)
    out[:, :N, :N] = tok
    out[:, NP - 1, :N] = bias[:, :, 0]                   # null key col
    out[:, NP - 1, N:] = 0.0                             # padded queries -> null only
    return out


def _host_prep(inputs):
    f = np.float32
    x = np.asarray(inputs['x'], f)
    Wq = np.asarray(inputs['Wq'], f)
    Wkv = np.asarray(inputs['Wkv'], f)
    bkv = np.asarray(inputs['bkv'], f)
    null_kv = np.asarray(inputs['null_kv'], f)
    Wo = np.asarray(inputs['Wo'], f)
    Wff1 = np.asarray(inputs['Wff1'], f)
    Wff2 = np.asarray(inputs['Wff2'], f)
    relpos_emb = np.asarray(inputs['relpos_emb'], f)
    Wproj = np.asarray(inputs['Wproj'], f)

    d = {}
    xT = np.zeros((B, D, NP), f)
    xT[:, :, :N] = np.transpose(x, (0, 2, 1))
    d['xT'] = xT

    # permuted Wq: [L, 8(head), 8(kchunk), 128, 64]
    Wq_p = Wq.reshape(L, D, H, DH)[:, :, :, PERM]        # [L, D, H, DH]
    d['wq'] = np.ascontiguousarray(
        Wq_p.reshape(L, 8, 128, H, DH).transpose(0, 3, 1, 2, 4))
    # permuted Wk: [L, 8, 128, 64]
    d['wk'] = np.ascontiguousarray(
        Wkv[:, :, :DH][:, :, PERM].reshape(L, 8, 128, DH))
    # Wv: [L, 8, 128, 64]
    d['wv'] = np.ascontiguousarray(Wkv[:, :, DH:].reshape(L, 8, 128, DH))
    d['bvv'] = np.ascontiguousarray(bkv[:, DH:].reshape(L, 1, DH))
    d['bk'] = np.ascontiguousarray(bkv[:, :DH][:, PERM].reshape(L, 1, DH))  # [L,1,64]
    d['ones1xNP'] = np.ones((1, NP), f)
    d['nullk'] = np.ascontiguousarray(null_kv[:, 0, PERM][:, :, None])  # [L,64,1]
    d['nullv'] = np.ascontiguousarray(null_kv[:, 1, :][:, None, :])     # [L,1,64]
    # Wo: [L, 8(mt), 8(kchunk of 64), 64, 128]
    d['wo'] = np.ascontiguousarray(
        Wo.reshape(L, 8, 64, 8, 128).transpose(0, 3, 1, 2, 4))
    # Wff1: [L, 32(m), 2(a/g), 8(k), 128, 128]
    d['wff1'] = np.ascontiguousarray(
        Wff1.reshape(L, 8, 128, 2, 32, 128).transpose(0, 4, 3, 1, 2, 5))
    # Wff2 (bf16): [L, 8(mt), 32(k), 128, 128]
    d['wff2'] = np.ascontiguousarray(
        Wff2.reshape(L, 32, 128, 8, 128).transpose(0, 3, 1, 2, 4)
    ).astype(ml_dtypes.bfloat16)
    # Wproj: [8(k), 2(half), 128, 512]
    d['wproj'] = np.ascontiguousarray(
        Wproj.reshape(8, 128, 2, 512).transpose(0, 2, 1, 3))
    gains = np.zeros((13, 8, 128), f)
    for l in range(L):
        gains[l * 3 + 0] = np.asarray(inputs['attn_norm_g'], f)[l].reshape(8, 128)
        gains[l * 3 + 1] = np.asarray(inputs['out_norm_g'], f)[l].reshape(8, 128)
        gains[l * 3 + 2] = np.asarray(inputs['ff_norm_g'], f)[l].reshape(8, 128)
    gains[12] = np.asarray(inputs['final_norm_g'], f).reshape(8, 128)
    d['gains'] = np.ascontiguousarray(gains.reshape(13 * 8, 128).T)  # [128, 104]
    inv_freq = (1.0 / (10000.0 ** (np.arange(0, ROT, 2, dtype=f) / ROT))).astype(f)
    freqs = np.arange(NP, dtype=f)[None, :] * inv_freq[:, None]
    d['cos'] = np.cos(freqs).astype(f)
    d['sin'] = np.sin(freqs).astype(f)
    d['biasT'] = _np_relpos_bias_T(relpos_emb).astype(ml_dtypes.bfloat16)
    d['ones64c'] = np.ones((64, 1), f)
    d['ones1x64'] = np.ones((1, 64), f)
    d['sixt1x64'] = np.full((1, 64), 16.0, f)
    d['ones1x128'] = np.ones((1, 128), f)
    d['ones128c'] = np.ones((128, 1), f)
    return d


def _build():
    nc = bacc.Bacc("TRN2", target_bir_lowering=False, debug=False, num_devices=8)

    def P(name, shape, dt=f32):
        return nc.declare_dram_parameter(name, list(shape), dt, isOutput=False)

    xT_d = P('xT', [D, NP])
    wq_d = P('wq', [L, H, 8, 128, DH])
    wk_d = P('wk', [L, 8, 128, DH])
    wv_d = P('wv', [L, 8, 128, DH])
    bvv_d = P('bvv', [L, 1, DH])
    bk_d = P('bk', [L, 1, DH])
    ones1xNP_d = P('ones1xNP', [1, NP])
    nullk_d = P('nullk', [L, DH, 1])
    nullv_d = P('nullv', [L, 1, DH])
    wo_d = P('wo', [L, 8, 8, 64, 128])
    wff1_d = P('wff1', [L, 32, 2, 8, 128, 128])
    wff2_d = P('wff2', [L, 8, 32, 128, 128], bf16)
    wproj_d = P('wproj', [8, 2, 128, 512])
    gains_d = P('gains', [128, 13 * 8])
    cos_d = P('cos', [16, NP])
    sin_d = P('sin', [16, NP])
    biasT_d = P('biasT', [H, NP, NP], bf16)
    ones64c_d = P('ones64c', [64, 1])
    ones1x64_d = P('ones1x64', [1, 64])
    sixt1x64_d = P('sixt1x64', [1, 64])
    ones1x128_d = P('ones1x128', [1, 128])
    ones128c_d = P('ones128c', [128, 1])
    out_d = nc.declare_dram_parameter('out', [N, D], f32, isOutput=True)

    R = f32r

    with nc.allow_low_precision("f32r data path; fp32 accumulation in PSUM"), \
         tile.TileContext(nc) as tc:
        with tc.tile_pool(name="const", bufs=1) as cpool, \
             tc.tile_pool(name="res", bufs=1) as rpool, \
             tc.tile_pool(name="wq", bufs=2) as wqp, \
             tc.tile_pool(name="wsmall", bufs=2) as wsp, \
             tc.tile_pool(name="wff1", bufs=3) as wf1p, \
             tc.tile_pool(name="wff2", bufs=4) as wf2p, \
             tc.tile_pool(name="scr", bufs=2) as scrp, \
             tc.tile_pool(name="attn", bufs=4) as attp, \
             tc.tile_pool(name="vec", bufs=2) as vecp, \
             tc.tile_pool(name="ps", bufs=3, space="PSUM") as ps, \
             tc.tile_pool(name="psacc", bufs=2, space="PSUM") as psacc, \
             tc.tile_pool(name="psbc", bufs=3, space="PSUM") as psbc:

            # ---------- constants ----------
            cos_t = cpool.tile([16, NP], f32, tag="cos")
            sin_t = cpool.tile([16, NP], f32, tag="sin")
            nc.gpsimd.dma_start(cos_t[:], cos_d[:])
            nc.gpsimd.dma_start(sin_t[:], sin_d[:])
            gains_t = cpool.tile([128, 13 * 8], f32, tag="gains")
            nc.gpsimd.dma_start(gains_t[:], gains_d[:])
            ones64c_t = cpool.tile([64, 1], R, tag="ones64c")
            nc.gpsimd.dma_start(ones64c_t[:], ones64c_d[:].bitcast(R))
            ones1x64_t = cpool.tile([1, 64], R, tag="ones1x64")
            nc.gpsimd.dma_start(ones1x64_t[:], ones1x64_d[:].bitcast(R))
            sixt1x64_t = cpool.tile([1, 64], R, tag="sixt1x64")
            nc.gpsimd.dma_start(sixt1x64_t[:], sixt1x64_d[:].bitcast(R))
            ones1x128_t = cpool.tile([1, 128], R, tag="ones1x128")
            nc.gpsimd.dma_start(ones1x128_t[:], ones1x128_d[:].bitcast(R))
            ones128c_t = cpool.tile([128, 1], R, tag="ones128c")
            nc.gpsimd.dma_start(ones128c_t[:], ones128c_d[:].bitcast(R))
            ones1xNP_t = cpool.tile([1, NP], R, tag="ones1xNP")
            nc.gpsimd.dma_start(ones1xNP_t[:], ones1xNP_d[:].bitcast(R))
            epsc = cpool.tile([128, 1], f32, tag="epsc")
            nc.gpsimd.memset(epsc[:], EPS)
            eps12 = cpool.tile([128, 1], f32, tag="eps12")
            nc.gpsimd.memset(eps12[:], 1e-12)

            def gcol(idx, mt):
                off = idx * 8 + mt
                return gains_t[:, off:off + 1]

            # ---------- persistent activations ----------
            xt = rpool.tile([128, NMT * NP], R, tag="x")
            nc.gpsimd.dma_start(
                xt[:].rearrange("p (c n) -> p c n", c=NMT),
                xT_d[:].rearrange("(c p) n -> p c n", p=128).bitcast(R))
            xn = rpool.tile([128, NMT * NP], R, tag="xn")
            qhat = rpool.tile([64, H * NP], R, tag="qhat")
            oT = rpool.tile([64, H * NP], R, tag="oT")
            khat = rpool.tile([64, NP], R, tag="khat")
            vaug = rpool.tile([128, 5 * DH], R, tag="vaug")
            sff = rpool.tile([128, 8 * NP], bf16, tag="sff")

            QTS = [(0, QT), (QT, QT)]

            def layer_norm_apply(src_tile, gidx, dst, residual):
                for (qo, qw) in QTS:
                    s1p = ps.tile([1, QT], f32, tag="mm")
                    s2p = ps.tile([1, QT], f32, tag="mm")
                    for mt in range(NMT):
                        seg = src_tile[:, mt * NP + qo:mt * NP + qo + qw]
                        sq = scrp.tile([128, QT], R, tag="sq")
                        nc.scalar.activation(sq[:], seg, AF.Square)
                        nc.tensor.matmul(s1p[:], ones128c_t[:], seg,
                                         start=(mt == 0), stop=(mt == NMT - 1))
                        nc.tensor.matmul(s2p[:], ones128c_t[:], sq[:],
                                         start=(mt == 0), stop=(mt == NMT - 1))
                    m_v = vecp.tile([1, QT], f32, tag="m")
                    nc.scalar.activation(m_v[:], s1p[:], AF.Copy, scale=1.0 / D)
                    q2_v = vecp.tile([1, QT], f32, tag="q2")
                    nc.scalar.activation(q2_v[:], s2p[:], AF.Copy, scale=1.0 / D)
                    msq_v = vecp.tile([1, QT], f32, tag="msq")
                    nc.scalar.activation(msq_v[:], m_v[:], AF.Square)
                    v_v = vecp.tile([1, QT], f32, tag="v")
                    nc.vector.tensor_sub(v_v[:], q2_v[:], msq_v[:])
                    s_v = vecp.tile([1, QT], f32, tag="s")
                    nc.scalar.activation(s_v[:], v_v[:], AF.Sqrt, bias=epsc[0:1, :])
                    r_v = vecp.tile([1, QT], R, tag="r")
                    nc.vector.reciprocal(r_v[:], s_v[:])
                    mr_v = vecp.tile([1, QT], R, tag="mr")
                    nc.vector.tensor_mul(mr_v[:], m_v[:], r_v[:])
                    rb = psbc.tile([128, QT], f32, tag="bc")
                    nc.tensor.matmul(rb[:], ones1x128_t[:], r_v[:], start=True, stop=True)
                    mrb = psbc.tile([128, QT], f32, tag="bc")
                    nc.tensor.matmul(mrb[:], ones1x128_t[:], mr_v[:], start=True, stop=True)
                    for mt in range(NMT):
                        seg = src_tile[:, mt * NP + qo:mt * NP + qo + qw]
                        t1 = scrp.tile([128, QT], f32, tag="t1")
                        nc.vector.tensor_mul(t1[:], seg, rb[:])
                        nc.vector.tensor_sub(t1[:], t1[:], mrb[:])
                        if residual:
                            t2 = scrp.tile([128, QT], f32, tag="t2")
                            nc.scalar.activation(t2[:], t1[:], AF.Copy,
                                                 scale=gcol(gidx, mt))
                            xcols = xt[:, mt * NP + qo:mt * NP + qo + qw]
                            nc.vector.tensor_add(xcols, xcols, t2[:])
                        else:
                            nc.scalar.activation(dst[:, mt * NP + qo:mt * NP + qo + qw],
                                                 t1[:], AF.Copy, scale=gcol(gidx, mt))

            def rot_norm(pq, dst_tile, dst_off, qo, qw, scale16):
                """rotary + l2norm (optionally *16) for one 64-row head psum."""
                cs = cos_t[:, qo:qo + qw]
                sn = sin_t[:, qo:qo + qw]
                qr = scrp.tile([64, QT], f32, tag="qr")
                nc.scalar.copy(qr[:], pq[:, :])
                tE = scrp.tile([16, QT], f32, tag="tE")
                tO = scrp.tile([16, QT], f32, tag="tO")
                nc.vector.tensor_mul(tE[:], pq[0:16, :], cs)
                nc.vector.tensor_mul(tO[:], pq[32:48, :], sn)
                nc.vector.tensor_sub(qr[0:16, :], tE[:], tO[:])
                tA = scrp.tile([16, QT], f32, tag="tE")
                tB = scrp.tile([16, QT], f32, tag="tO")
                nc.vector.tensor_mul(tA[:], pq[32:48, :], cs)
                nc.vector.tensor_mul(tB[:], pq[0:16, :], sn)
                nc.vector.tensor_add(qr[32:48, :], tA[:], tB[:])
                sq = scrp.tile([64, QT], R, tag="hsq")
                nc.scalar.activation(sq[:], qr[:], AF.Square)
                ssq = ps.tile([1, QT], f32, tag="mm")
                nc.tensor.matmul(ssq[:], ones64c_t[:], sq[:], start=True, stop=True)
                sh = vecp.tile([1, QT], f32, tag="sh")
                nc.scalar.activation(sh[:], ssq[:], AF.Sqrt, bias=eps12[0:1, :])
                rh = vecp.tile([1, QT], R, tag="rh")
                nc.vector.reciprocal(rh[:], sh[:])
                bcq = psbc.tile([64, QT], f32, tag="bc")
                nc.tensor.matmul(bcq[:], sixt1x64_t[:] if scale16 else ones1x64_t[:],
                                 rh[:], start=True, stop=True)
                nc.vector.tensor_mul(dst_tile[0:64, dst_off + qo:dst_off + qo + qw],
                                     qr[:], bcq[:])

            # ================= layers =================
            for l in range(L):
                layer_norm_apply(xt, l * 3 + 0, xn, residual=False)

                # ---- Q per head ----
                for h in range(H):
                    wqt = wqp.tile([128, 8 * DH], R, tag="wq")
                    nc.gpsimd.dma_start(
                        wqt[:].rearrange("p (c m) -> p c m", c=8),
                        wq_d[l, h].rearrange("c p m -> p c m").bitcast(R))
                    for (qo, qw) in QTS:
                        pq = ps.tile([64, QT], f32, tag="mm")
                        for c in range(8):
                            nc.tensor.matmul(pq[:], wqt[:, c * DH:(c + 1) * DH],
                                             xn[:, c * NP + qo:c * NP + qo + qw],
                                             start=(c == 0), stop=(c == 7))
                        rot_norm(pq, qhat, h * NP, qo, qw, scale16=True)

                # ---- K ----
                wkt = wsp.tile([128, 8 * DH], R, tag="wk")
                nc.gpsimd.dma_start(
                    wkt[:].rearrange("p (c m) -> p c m", c=8),
                    wk_d[l].rearrange("c p m -> p c m").bitcast(R))
                bkr = wsp.tile([1, DH], R, tag="bkr")
                nc.gpsimd.dma_start(bkr[:], bk_d[l].bitcast(R))
                ks = scrp.tile([64, NP], f32, tag="ks")
                for (qo, qw) in QTS:
                    pk = ps.tile([64, QT], f32, tag="mm")
                    for c in range(8):
                        nc.tensor.matmul(pk[:], wkt[:, c * DH:(c + 1) * DH],
                                         xn[:, c * NP + qo:c * NP + qo + qw],
                                         start=(c == 0), stop=False)
                    nc.tensor.matmul(pk[:], bkr[:], ones1xNP_t[:, qo:qo + qw],
                                     start=False, stop=True)
                    cs = cos_t[:, qo:qo + qw]
                    sn = sin_t[:, qo:qo + qw]
                    nc.scalar.copy(ks[:, qo:qo + qw], pk[:, :])
                    tE = scrp.tile([16, QT], f32, tag="tE")
                    tO = scrp.tile([16, QT], f32, tag="tO")
                    nc.vector.tensor_mul(tE[:], pk[0:16, :], cs)
                    nc.vector.tensor_mul(tO[:], pk[32:48, :], sn)
                    nc.vector.tensor_sub(ks[0:16, qo:qo + qw], tE[:], tO[:])
                    tA = scrp.tile([16, QT], f32, tag="tE")
                    tB = scrp.tile([16, QT], f32, tag="tO")
                    nc.vector.tensor_mul(tA[:], pk[32:48, :], cs)
                    nc.vector.tensor_mul(tB[:], pk[0:16, :], sn)
                    nc.vector.tensor_add(ks[32:48, qo:qo + qw], tA[:], tB[:])
                nc.gpsimd.dma_start(ks[:, NP - 1:NP], nullk_d[l])
                for (qo, qw) in QTS:
                    sq = scrp.tile([64, QT], R, tag="hsq")
                    nc.scalar.activation(sq[:], ks[:, qo:qo + qw], AF.Square)
                    ssq = ps.tile([1, QT], f32, tag="mm")
                    nc.tensor.matmul(ssq[:], ones64c_t[:], sq[:], start=True, stop=True)
                    sh = vecp.tile([1, QT], f32, tag="sh")
                    nc.scalar.activation(sh[:], ssq[:], AF.Sqrt, bias=eps12[0:1, :])
                    rh = vecp.tile([1, QT], R, tag="rh")
                    nc.vector.reciprocal(rh[:], sh[:])
                    bck = psbc.tile([64, QT], f32, tag="bc")
                    nc.tensor.matmul(bck[:], ones1x64_t[:], rh[:], start=True, stop=True)
                    nc.vector.tensor_mul(khat[:, qo:qo + qw], ks[:, qo:qo + qw], bck[:])

                # ---- V (natural layout) ----
                wvt = wsp.tile([128, 8 * DH], R, tag="wv")
                nc.gpsimd.dma_start(
                    wvt[:].rearrange("p (c m) -> p c m", c=8),
                    wv_d[l].rearrange("c p m -> p c m").bitcast(R))
                bvt = wsp.tile([1, DH], R, tag="bvv")
                nc.gpsimd.dma_start(bvt[:], bvv_d[l].bitcast(R))
                for t, (to, tw) in enumerate(TCH):
                    pv = ps.tile([128, DH], f32, tag="mm")
                    for c in range(8):
                        nc.tensor.matmul(pv[0:tw, :], xn[:, c * NP + to:c * NP + to + tw],
                                         wvt[:, c * DH:(c + 1) * DH],
                                         start=(c == 0), stop=False)
                    nc.tensor.matmul(pv[0:tw, :], ones1x128_t[:, 0:tw], bvt[:],
                                     start=False, stop=True)
                    nc.vector.tensor_copy(vaug[0:tw, t * DH:(t + 1) * DH], pv[0:tw, :])
                nc.gpsimd.dma_start(vaug[7:8, 4 * DH:5 * DH], nullv_d[l].bitcast(R))

                # ---- attention ----
                for h in range(H):
                    for (qo, qw) in QTS:
                        av = psacc.tile([64, QT], f32, tag="acc")
                        dsum = psbc.tile([1, QT], f32, tag="bc")
                        for c, (ko, kw) in enumerate(KCH):
                            sp = ps.tile([128, QT], f32, tag="mm")
                            nc.tensor.matmul(sp[0:kw, :], khat[:, ko:ko + kw],
                                             qhat[:, h * NP + qo:h * NP + qo + qw],
                                             start=True, stop=True)
                            bt = attp.tile([128, QT], bf16, tag="bias")
                            nc.gpsimd.dma_start(bt[0:kw, :], biasT_d[h, ko:ko + kw, qo:qo + qw])
                            sc = attp.tile([128, QT], f32, tag="sc")
                            nc.vector.scalar_tensor_tensor(sc[0:kw, :], sp[0:kw, :], 1.0,
                                                           bt[0:kw, :], ALU.mult, ALU.add)
                            au = attp.tile([128, QT], R, tag="au")
                            nc.scalar.activation(au[0:kw, :], sc[0:kw, :], AF.Exp)
                            nc.tensor.matmul(av[:], vaug[0:kw, c * DH:(c + 1) * DH],
                                             au[0:kw, :], start=(c == 0), stop=(c == 4))
                            nc.tensor.matmul(dsum[:], ones128c_t[0:kw, :],
                                             au[0:kw, :], start=(c == 0), stop=(c == 4))
                        rd = vecp.tile([1, QT], R, tag="rd")
                        nc.vector.reciprocal(rd[:], dsum[:])
                        bco = psbc.tile([64, QT], f32, tag="bc")
                        nc.tensor.matmul(bco[:], ones1x64_t[:], rd[:], start=True, stop=True)
                        oc = attp.tile([64, QT], f32, tag="oc")
                        nc.scalar.copy(oc[:], av[:])
                        nc.vector.tensor_mul(oT[0:64, h * NP + qo:h * NP + qo + qw],
                                             oc[:], bco[:])

                # ---- Wo -> o2 (into xn) + LN2 + residual ----
                o2 = xn
                for mt in range(NMT):
                    wot = wsp.tile([64, 8 * 128], R, tag="wo")
                    nc.gpsimd.dma_start(
                        wot[:].rearrange("p (c m) -> p c m", c=8),
                        wo_d[l, mt].rearrange("c p m -> p c m").bitcast(R))
                    for (qo, qw) in QTS:
                        pl = ps.tile([128, QT], f32, tag="mm")
                        for c in range(8):
                            nc.tensor.matmul(pl[:], wot[:, c * 128:(c + 1) * 128],
                                             oT[:, c * NP + qo:c * NP + qo + qw],
                                             start=(c == 0), stop=(c == 7))
                        nc.scalar.activation(o2[:, mt * NP + qo:mt * NP + qo + qw],
                                             pl[:], AF.Copy)
                layer_norm_apply(o2, l * 3 + 1, None, residual=True)

                layer_norm_apply(xt, l * 3 + 2, xn, residual=False)

                # ---- FFN in four quarters of 8 s-blocks ----
                for half in range(4):
                    for mi in range(8):
                        m = half * 8 + mi
                        wga = wf1p.tile([128, 8 * 128], R, tag="wff1")
                        nc.gpsimd.dma_start(
                            wga[:].rearrange("p (c m) -> p c m", c=8),
                            wff1_d[l, m, 0].rearrange("c p m -> p c m").bitcast(R))
                        wgg = wf1p.tile([128, 8 * 128], R, tag="wff1")
                        nc.gpsimd.dma_start(
                            wgg[:].rearrange("p (c m) -> p c m", c=8),
                            wff1_d[l, m, 1].rearrange("c p m -> p c m").bitcast(R))
                        for (qo, qw) in QTS:
                            pg = ps.tile([128, QT], f32, tag="mm")
                            for c in range(8):
                                nc.tensor.matmul(pg[:], wgg[:, c * 128:(c + 1) * 128],
                                                 xn[:, c * NP + qo:c * NP + qo + qw],
                                                 start=(c == 0), stop=(c == 7))
                            sg = attp.tile([128, QT], bf16, tag="sg")
                            if os.environ.get('KSIM'):
                                sig = attp.tile([128, QT], f32, tag="sig")
                                nc.scalar.activation(sig[:], pg[:], AF.Sigmoid)
                                nc.vector.tensor_mul(sg[:], pg[:], sig[:])
                            else:
                                nc.scalar.activation(sg[:], pg[:], AF.Silu)
                            pa = ps.tile([128, QT], f32, tag="mm")
                            for c in range(8):
                                nc.tensor.matmul(pa[:], wga[:, c * 128:(c + 1) * 128],
                                                 xn[:, c * NP + qo:c * NP + qo + qw],
                                                 start=(c == 0), stop=(c == 7))
                            nc.vector.tensor_mul(sff[:, mi * NP + qo:mi * NP + qo + qw],
                                                 pa[:], sg[:])
                    for mt in range(NMT):
                        w2 = wf2p.tile([128, 8 * 128], bf16, tag="wff2")
                        nc.gpsimd.dma_start(
                            w2[:].rearrange("p (c m) -> p c m", c=8),
                            wff2_d[l, mt, half * 8:(half + 1) * 8]
                            .rearrange("c p m -> p c m"))
                        for (qo, qw) in QTS:
                            pl = ps.tile([128, QT], f32, tag="mm")
                            for c in range(8):
                                nc.tensor.matmul(pl[:], w2[:, c * 128:(c + 1) * 128],
                                                 sff[:, c * NP + qo:c * NP + qo + qw],
                                                 start=(c == 0), stop=(c == 7))
                            xcols = xt[:, mt * NP + qo:mt * NP + qo + qw]
                            nc.vector.tensor_add(xcols, xcols, pl[:])

            # ================= final stable LN + Wproj =================
            xm = cpool.tile([128, NP], f32, tag="xm")
            nc.vector.tensor_max(xm[:], xt[:, 0:NP], xt[:, NP:2 * NP])
            for mt in range(2, NMT):
                nc.vector.tensor_max(xm[:], xm[:], xt[:, mt * NP:(mt + 1) * NP])
            mxb = cpool.tile([128, NP], f32, tag="mxb")
            nc.gpsimd.partition_all_reduce(mxb[:], xm[:], 128, bass_isa.ReduceOp.max)

            for (qo, qw) in QTS:
                s1p = ps.tile([1, QT], f32, tag="mm")
                s2p = ps.tile([1, QT], f32, tag="mm")
                for mt in range(NMT):
                    seg = xt[:, mt * NP + qo:mt * NP + qo + qw]
                    sq = scrp.tile([128, QT], R, tag="sq")
                    nc.scalar.activation(sq[:], seg, AF.Square)
                    nc.tensor.matmul(s1p[:], ones128c_t[:], seg,
                                     start=(mt == 0), stop=(mt == NMT - 1))
                    nc.tensor.matmul(s2p[:], ones128c_t[:], sq[:],
                                     start=(mt == 0), stop=(mt == NMT - 1))
                m_v = vecp.tile([1, QT], f32, tag="m")
                nc.scalar.activation(m_v[:], s1p[:], AF.Copy, scale=1.0 / D)
                q2_v = vecp.tile([1, QT], f32, tag="q2")
                nc.scalar.activation(q2_v[:], s2p[:], AF.Copy, scale=1.0 / D)
                msq_v = vecp.tile([1, QT], f32, tag="msq")
                nc.scalar.activation(msq_v[:], m_v[:], AF.Square)
                v_v = vecp.tile([1, QT], f32, tag="v")
                nc.vector.tensor_sub(v_v[:], q2_v[:], msq_v[:])
                mxsq_v = vecp.tile([1, QT], f32, tag="mxsq")
                nc.scalar.activation(mxsq_v[:], mxb[0:1, qo:qo + qw], AF.Square)
                veps_v = vecp.tile([1, QT], f32, tag="veps")
                nc.vector.scalar_tensor_tensor(veps_v[:], mxsq_v[:], EPS, v_v[:],
                                               ALU.mult, ALU.add)
                s_v = vecp.tile([1, QT], f32, tag="s")
                nc.scalar.activation(s_v[:], veps_v[:], AF.Sqrt)
                r_v = vecp.tile([1, QT], R, tag="r")
                nc.vector.reciprocal(r_v[:], s_v[:])
                mr_v = vecp.tile([1, QT], R, tag="mr")
                nc.vector.tensor_mul(mr_v[:], m_v[:], r_v[:])
                rb = psbc.tile([128, QT], f32, tag="bc")
                nc.tensor.matmul(rb[:], ones1x128_t[:], r_v[:], start=True, stop=True)
                mrb = psbc.tile([128, QT], f32, tag="bc")
                nc.tensor.matmul(mrb[:], ones1x128_t[:], mr_v[:], start=True, stop=True)
                for mt in range(NMT):
                    t1 = scrp.tile([128, QT], f32, tag="t1")
                    nc.vector.tensor_mul(t1[:], xt[:, mt * NP + qo:mt * NP + qo + qw], rb[:])
                    nc.vector.tensor_sub(t1[:], t1[:], mrb[:])
                    nc.scalar.activation(xn[:, mt * NP + qo:mt * NP + qo + qw],
                                         t1[:], AF.Copy, scale=gcol(12, mt))

            for t, (to, tw) in enumerate(TCH):
                rtw = min(tw, max(0, N - to))
                if rtw == 0:
                    continue
                for half in range(2):
                    pn = psacc.tile([128, 512], f32, tag="acc")
                    for c in range(8):
                        wpt = wqp.tile([128, 512], R, tag="wproj")
                        nc.gpsimd.dma_start(wpt[:], wproj_d[c, half].bitcast(R))
                        nc.tensor.matmul(pn[0:tw, :], xn[:, c * NP + to:c * NP + to + tw],
                                         wpt[:], start=(c == 0), stop=(c == 7))
                    st = scrp.tile([128, 512], f32, tag="outst")
                    nc.vector.tensor_copy(st[0:rtw, :], pn[0:rtw, :])
                    nc.gpsimd.dma_start(out_d[to:to + rtw, half * 512:(half + 1) * 512],
                                        st[0:rtw, :])

    nc.compile()
    return nc


_CACHE = {}


def _get_program():
    if 'nc' not in _CACHE:
        _CACHE['nc'] = _build()
    return _CACHE['nc']


def kernel(**inputs) -> np.ndarray:
    from concourse.bass_utils import run_bass_kernel_spmd
    host = _host_prep(inputs)
    nc = _get_program()
    shared = {k: v for k, v in host.items() if k != 'xT'}
    in_maps = [dict(shared, xT=np.ascontiguousarray(host['xT'][b])) for b in range(B)]
    res = run_bass_kernel_spmd(nc, in_maps, list(range(B)))
    out = np.stack([res.results[b]['out'] for b in range(B)], axis=0)
    _CACHE['last_results'] = res
    return out

